# revision 8
# baseline (speedup 1.0000x reference)
"""ClusterLoss Bass/Tile kernel for Trainium2 (8 NeuronCores, data parallel).

Strategy: pure data parallelism over the batch dim B=2048 -> 256 samples per
core.  Per core, samples are processed in two 128-partition blocks with the
sample index on partitions and the feature dim d on the free axis.  The
batched mat-vec einsum('bd,bnd->bn') is computed with one fused DVE
tensor_tensor_reduce (multiply + free-axis sum) per negative index n, reading
hn tiles of shape (128 samples, 8 negatives, 512 d) streamed from HBM.  The
masked logsumexp, cross-entropy, and BML terms are small per-partition vector
ops.  Each core reduces its per-sample contributions over partitions with a
single ones-vector matmul and writes 5 partial sums; the final scalar combine
runs on host (the only cross-core communication needed).
"""

from contextlib import ExitStack

import numpy as np

import concourse.bacc as bacc
import concourse.tile as tile
from concourse import mybir
from concourse import bass_utils

N_CORES = 8
B, D, N_MAX, M_MAX = 2048, 512, 256, 32
B_LOC = B // N_CORES          # 256 samples per core
PBLK = 128                    # partition block
NBLK = B_LOC // PBLK          # 2 blocks per core
NCHUNK = 8                    # negatives per hn DMA tile

TEMP, ALPHA, BETA, LAMBDA_BML = 0.07, 0.4, 0.2, 0.2
NEG = -1e30
EXP_CLAMP = -87.0             # exp(-87) underflows f32; avoids LUT extremes

F32 = mybir.dt.float32
I32 = mybir.dt.int32
AF = mybir.ActivationFunctionType
OP = mybir.AluOpType
AX = mybir.AxisListType


def _emit(tc, q, k, k2, hn, fn, hc, fc, out):
    nc = tc.nc
    with ExitStack() as ctx:
        hpool = ctx.enter_context(tc.tile_pool(name="hnp", bufs=5))
        fpool = ctx.enter_context(tc.tile_pool(name="fnp", bufs=2))
        qpool = ctx.enter_context(tc.tile_pool(name="qkp", bufs=2))
        mpool = ctx.enter_context(tc.tile_pool(name="med", bufs=2))
        spool = ctx.enter_context(tc.tile_pool(name="scr", bufs=2))
        smpool = ctx.enter_context(tc.tile_pool(name="sm", bufs=2))
        cpool = ctx.enter_context(tc.tile_pool(name="cst", bufs=1))
        ppool = ctx.enter_context(tc.tile_pool(name="ps", bufs=2, space="PSUM"))

        # constants
        iota_i = cpool.tile([PBLK, N_MAX], I32, tag="iota_i", name="iota_i")
        nc.gpsimd.iota(iota_i[:], pattern=[[1, N_MAX]], base=0, channel_multiplier=0)
        iota_f = cpool.tile([PBLK, N_MAX], F32, tag="iota_f", name="iota_f")
        nc.vector.tensor_copy(out=iota_f[:], in_=iota_i[:])
        ones = cpool.tile([PBLK, 1], F32, tag="ones", name="ones")
        nc.vector.memset(ones[:], 1.0)
        alpha_t = cpool.tile([PBLK, 1], F32, tag="alpha_t", name="alpha_t")
        nc.vector.memset(alpha_t[:], ALPHA)
        nbeta_t = cpool.tile([PBLK, 1], F32, tag="nbeta_t", name="nbeta_t")
        nc.vector.memset(nbeta_t[:], -BETA)

        def sm(tag, dt=F32, w=1):
            return smpool.tile([PBLK, w], dt, tag=tag, name=tag)

        blk_contribs = []
        for b in range(NBLK):
            q_t = qpool.tile([PBLK, D], F32, tag="q_t", name="q_t")
            nc.sync.dma_start(out=q_t[:], in_=q[b])
            k_t = qpool.tile([PBLK, D], F32, tag="k_t", name="k_t")
            nc.sync.dma_start(out=k_t[:], in_=k[b])
            k2_t = qpool.tile([PBLK, D], F32, tag="k2_t", name="k2_t")
            nc.sync.dma_start(out=k2_t[:], in_=k2[b])
            hc_i = sm("hc_i", I32)
            nc.sync.dma_start(out=hc_i[:], in_=hc[b])
            fc_i = sm("fc_i", I32)
            nc.sync.dma_start(out=fc_i[:], in_=fc[b])
            hc_f = sm("hc_f")
            nc.vector.tensor_copy(out=hc_f[:], in_=hc_i[:])
            fc_f = sm("fc_f")
            nc.vector.tensor_copy(out=fc_f[:], in_=fc_i[:])

            scr = spool.tile([PBLK, D], F32, tag="scr", name="scr")  # ttr product dump

            def ttr(in0, in1, scale, accum):
                # fused (in0*scale)*in1 with free-axis sum into accum (one
                # DVE pass); native InstTensorScalarPtr, unlike
                # tensor_tensor_reduce whose raw-ISA encoding this runtime
                # rejects
                nc.vector.scalar_tensor_tensor(
                    out=scr[:], in0=in0, scalar=scale, in1=in1,
                    op0=OP.mult, op1=OP.mult, accum_out=accum,
                )

            lpos = sm("lpos")
            ttr(q_t[:], k_t[:], 1.0 / TEMP, lpos[:])
            lposnb = sm("lposnb")
            ttr(q_t[:], k2_t[:], 1.0 / TEMP, lposnb[:])
            simpos = sm("simpos")
            ttr(q_t[:], k_t[:], 1.0, simpos[:])

            # negative logits: l_neg[b, n] = q . hn[b, n] / TEMP
            lneg = mpool.tile([PBLK, N_MAX], F32, tag="lneg", name="lneg")
            for c in range(N_MAX // NCHUNK):
                h_t = hpool.tile([PBLK, NCHUNK, D], F32, tag="h_t", name="h_t")
                nc.sync.dma_start(
                    out=h_t[:], in_=hn[b, :, c * NCHUNK:(c + 1) * NCHUNK, :]
                )
                for j in range(NCHUNK):
                    n = c * NCHUNK + j
                    ttr(h_t[:, j, :], q_t[:], 1.0 / TEMP, lneg[:, n:n + 1])

            # fn dots: q . fn[b, m]
            fnd = sm("fnd", w=M_MAX)
            for c in range(M_MAX // NCHUNK):
                f_t = fpool.tile([PBLK, NCHUNK, D], F32, tag="f_t", name="f_t")
                nc.sync.dma_start(
                    out=f_t[:], in_=fn[b, :, c * NCHUNK:(c + 1) * NCHUNK, :]
                )
                for j in range(NCHUNK):
                    m = c * NCHUNK + j
                    ttr(f_t[:, j, :], q_t[:], 1.0, fnd[:, m:m + 1])

            # mask padded negatives to -1e30, then logsumexp along free axis
            mneg = mpool.tile([PBLK, N_MAX], F32, tag="mneg", name="mneg")
            nc.vector.tensor_scalar(
                out=mneg[:], in0=iota_f[:], scalar1=hc_f[:], scalar2=NEG,
                op0=OP.is_ge, op1=OP.mult,
            )
            nc.vector.tensor_add(out=lneg[:], in0=lneg[:], in1=mneg[:])
            mrow = sm("mrow")
            nc.vector.tensor_reduce(out=mrow[:], in_=lneg[:], axis=AX.X, op=OP.max)
            nmrow = sm("nmrow")
            nc.vector.tensor_scalar_mul(out=nmrow[:], in0=mrow[:], scalar1=-1.0)
            expin = mpool.tile([PBLK, N_MAX], F32, tag="expin", name="expin")
            nc.vector.tensor_scalar(
                out=expin[:], in0=lneg[:], scalar1=nmrow[:], scalar2=EXP_CLAMP,
                op0=OP.add, op1=OP.max,
            )
            expout = mpool.tile([PBLK, N_MAX], F32, tag="expout", name="expout")
            sumexp = sm("sumexp")
            nc.scalar.activation(
                out=expout[:], in_=expin[:], func=AF.Exp, accum_out=sumexp[:]
            )
            lse = sm("lse")
            nc.scalar.activation(out=lse[:], in_=sumexp[:], func=AF.Ln)
            nc.vector.tensor_add(out=lse[:], in0=lse[:], in1=mrow[:])

            # ce(lp) = logaddexp(lp, lse) - lp
            def ce(lp, tag):
                mm = sm("mm" + tag)
                nc.vector.tensor_max(out=mm[:], in0=lp[:], in1=lse[:])
                nmm = sm("nmm" + tag)
                nc.vector.tensor_scalar_mul(out=nmm[:], in0=mm[:], scalar1=-1.0)
                e1 = sm("e1" + tag)
                nc.vector.tensor_scalar(
                    out=e1[:], in0=lp[:], scalar1=nmm[:], scalar2=EXP_CLAMP,
                    op0=OP.add, op1=OP.max,
                )
                nc.scalar.activation(out=e1[:], in_=e1[:], func=AF.Exp)
                e2 = sm("e2" + tag)
                nc.vector.tensor_scalar(
                    out=e2[:], in0=lse[:], scalar1=nmm[:], scalar2=EXP_CLAMP,
                    op0=OP.add, op1=OP.max,
                )
                nc.scalar.activation(out=e2[:], in_=e2[:], func=AF.Exp)
                s12 = sm("s12" + tag)
                nc.vector.tensor_add(out=s12[:], in0=e1[:], in1=e2[:])
                nc.scalar.activation(out=s12[:], in_=s12[:], func=AF.Ln)
                cev = sm("ce" + tag)
                nc.vector.tensor_add(out=cev[:], in0=s12[:], in1=mm[:])
                nc.vector.tensor_sub(out=cev[:], in0=cev[:], in1=lp[:])
                return cev

            cep = ce(lpos, "p")
            cenb = ce(lposnb, "n")

            # BML term
            maskf = sm("maskf", w=M_MAX)
            nc.vector.tensor_scalar(
                out=maskf[:], in0=iota_f[:, :M_MAX], scalar1=fc_f[:], scalar2=None,
                op0=OP.is_lt,
            )
            nc.vector.tensor_mul(out=fnd[:], in0=fnd[:], in1=maskf[:])
            sfn = sm("sfn")
            nc.vector.tensor_reduce(out=sfn[:], in_=fnd[:], axis=AX.X, op=OP.add)
            den = sm("den")
            nc.vector.tensor_scalar_max(out=den[:], in0=fc_f[:], scalar1=1.0)
            rden = sm("rden")
            nc.vector.reciprocal(out=rden[:], in_=den[:])
            simfn = sm("simfn")
            nc.vector.tensor_mul(out=simfn[:], in0=sfn[:], in1=rden[:])
            delta = sm("delta")
            nc.vector.tensor_sub(out=delta[:], in0=simfn[:], in1=simpos[:])
            r1 = sm("r1")
            nc.scalar.activation(out=r1[:], in_=delta[:], func=AF.Relu,
                                 bias=alpha_t[:], scale=1.0)
            r2 = sm("r2")
            nc.scalar.activation(out=r2[:], in_=delta[:], func=AF.Relu,
                                 bias=nbeta_t[:], scale=-1.0)
            bml = sm("bml")
            nc.vector.tensor_add(out=bml[:], in0=r1[:], in1=r2[:])

            vh = sm("vh")
            nc.vector.tensor_scalar(out=vh[:], in0=hc_f[:], scalar1=0.0,
                                    scalar2=None, op0=OP.is_gt)
            vf = sm("vf")
            nc.vector.tensor_scalar(out=vf[:], in0=fc_f[:], scalar1=0.0,
                                    scalar2=None, op0=OP.is_gt)
            vb = sm("vb")
            nc.vector.tensor_mul(out=vb[:], in0=vh[:], in1=vf[:])

            contrib = smpool.tile([PBLK, 5], F32, tag="contrib", name="contrib")
            nc.vector.tensor_mul(out=contrib[:, 0:1], in0=cep[:], in1=vh[:])
            nc.vector.tensor_mul(out=contrib[:, 1:2], in0=cenb[:], in1=vh[:])
            nc.vector.tensor_mul(out=contrib[:, 2:3], in0=bml[:], in1=vb[:])
            nc.vector.tensor_copy(out=contrib[:, 3:4], in_=vh[:])
            nc.vector.tensor_copy(out=contrib[:, 4:5], in_=vb[:])
            blk_contribs.append(contrib)

        tot = blk_contribs[0]
        nc.vector.tensor_add(out=tot[:], in0=tot[:], in1=blk_contribs[1][:])

        ps = ppool.tile([5, 1], F32, tag="ps5", name="ps5")
        nc.tensor.matmul(ps[:], lhsT=tot[:], rhs=ones[:], start=True, stop=True)
        res = smpool.tile([5, 1], F32, tag="res", name="res")
        nc.scalar.copy(out=res[:], in_=ps[:])
        nc.sync.dma_start(out=out[:], in_=res[:])


def _build():
    nc = bacc.Bacc("TRN2", target_bir_lowering=False, debug=False)
    q = nc.dram_tensor("q", [NBLK, PBLK, D], F32, kind="ExternalInput")
    k = nc.dram_tensor("k", [NBLK, PBLK, D], F32, kind="ExternalInput")
    k2 = nc.dram_tensor("k2", [NBLK, PBLK, D], F32, kind="ExternalInput")
    hn = nc.dram_tensor("hn", [NBLK, PBLK, N_MAX, D], F32, kind="ExternalInput")
    fn = nc.dram_tensor("fn", [NBLK, PBLK, M_MAX, D], F32, kind="ExternalInput")
    hc = nc.dram_tensor("hn_counts", [NBLK, PBLK, 1], I32, kind="ExternalInput")
    fc = nc.dram_tensor("fn_counts", [NBLK, PBLK, 1], I32, kind="ExternalInput")
    out = nc.dram_tensor("out", [5, 1], F32, kind="ExternalOutput")
    with tile.TileContext(nc) as tc:
        _emit(tc, q, k, k2, hn, fn, hc, fc, out)
    nc.compile()
    return nc


_NC_CACHE = []


def _get_nc():
    if not _NC_CACHE:
        _NC_CACHE.append(_build())
    return _NC_CACHE[0]


def make_in_maps(q, k, k2, hn, fn, hn_counts, fn_counts):
    q = np.asarray(q, np.float32)
    k = np.asarray(k, np.float32)
    k2 = np.asarray(k2, np.float32)
    hn = np.asarray(hn, np.float32)
    fn = np.asarray(fn, np.float32)
    hn_counts = np.asarray(hn_counts, np.int32)
    fn_counts = np.asarray(fn_counts, np.int32)
    in_maps = []
    for c in range(N_CORES):
        s = slice(c * B_LOC, (c + 1) * B_LOC)
        in_maps.append({
            "q": np.ascontiguousarray(q[s]).reshape(NBLK, PBLK, D),
            "k": np.ascontiguousarray(k[s]).reshape(NBLK, PBLK, D),
            "k2": np.ascontiguousarray(k2[s]).reshape(NBLK, PBLK, D),
            "hn": np.ascontiguousarray(hn[s]).reshape(NBLK, PBLK, N_MAX, D),
            "fn": np.ascontiguousarray(fn[s]).reshape(NBLK, PBLK, M_MAX, D),
            "hn_counts": np.ascontiguousarray(hn_counts[s]).reshape(NBLK, PBLK, 1),
            "fn_counts": np.ascontiguousarray(fn_counts[s]).reshape(NBLK, PBLK, 1),
        })
    return in_maps


def combine_partials(results):
    parts = np.stack([np.asarray(r["out"], np.float64).reshape(5) for r in results])
    cl_s, clnb_s, bml_s, nv, nb = parts.sum(axis=0)
    n_valid = max(nv, 1.0)
    cl = cl_s / n_valid
    clnb = clnb_s / n_valid
    bml_mean = (bml_s / nb) if nb > 0 else 0.0
    lbml = LAMBDA_BML * bml_mean
    tot = cl + clnb + lbml
    return np.array([tot, cl, lbml, clnb], np.float32)


def run_spmd(in_maps, **kwargs):
    nc = _get_nc()
    return bass_utils.run_bass_kernel_spmd(
        nc, in_maps, core_ids=list(range(N_CORES)), **kwargs
    )


def kernel(q, k, k2, hn, fn, hn_counts, fn_counts):
    in_maps = make_in_maps(q, k, k2, hn, fn, hn_counts, fn_counts)
    res = run_spmd(in_maps)
    return combine_partials(res.results)


# revision 10
# speedup vs baseline: 1.1943x; 1.1943x over previous
"""ClusterLoss Bass/Tile kernel for Trainium2 (8 NeuronCores, data parallel).

Strategy
--------
Pure data parallelism over B=2048 with a count-aware schedule: samples are
globally sorted by hn_count and dealt into 16 blocks of 128; every core gets
one "big" block (slot 0, padded negative bound N0) and one "small" block
(slot 1, bound N1), so a single SPMD program with static loop bounds fits all
cores while skipping most padded negatives.  The four losses are sums over
samples, so no output unpermutation is needed.

Per block, samples sit on the 128 partitions with the feature dim d on the
free axis.  The einsum('bd,bnd->bn') runs as one fused DVE tensor_tensor
multiply per 8-negative chunk (q broadcast via a stride-0 access pattern)
with the per-negative free-axis reductions on the otherwise idle Scalar
engine (activation Copy with accum_out, folding the 1/TEMP scale).  Masked
logsumexp / cross-entropy / BML terms are small per-partition vector ops.
Each core emits 5 partial sums reduced over partitions with a ones-vector
matmul; the final scalar combine runs on host.

The program is JIT-specialized to (N0, N1) derived from the counts at call
time and cached, so repeated calls with the same raggedness profile reuse
the compiled NEFF.
"""

from contextlib import ExitStack

import numpy as np

import concourse.bass as bass
import concourse.bacc as bacc
import concourse.tile as tile
from concourse import mybir
from concourse import bass_utils

N_CORES = 8
B, D, N_MAX, M_MAX = 2048, 512, 256, 32
B_LOC = B // N_CORES          # 256 samples per core
PBLK = 128                    # partition block
NBLK = B_LOC // PBLK          # 2 slots per core
NCHUNK = 8                    # negatives per hn DMA tile / DVE multiply

TEMP, ALPHA, BETA, LAMBDA_BML = 0.07, 0.4, 0.2, 0.2
NEG = -1e30
EXP_CLAMP = -87.0             # exp(-87) underflows f32; avoids LUT extremes

F32 = mybir.dt.float32
I32 = mybir.dt.int32
AF = mybir.ActivationFunctionType
OP = mybir.AluOpType
AX = mybir.AxisListType


def _bcast_n(ap, n):
    """(128, D) AP viewed as (128, n, D) with stride-0 broadcast on n."""
    return bass.AP(tensor=ap.tensor, offset=ap.offset,
                   ap=[ap.ap[0], [0, n], ap.ap[1]])


def _emit(tc, bounds, q, k, k2, hns, fn, hc, fc, out):
    nc = tc.nc
    with ExitStack() as ctx:
        hpool = ctx.enter_context(tc.tile_pool(name="hnp", bufs=4))
        fpool = ctx.enter_context(tc.tile_pool(name="fnp", bufs=2))
        qpool = ctx.enter_context(tc.tile_pool(name="qkp", bufs=2))
        mpool = ctx.enter_context(tc.tile_pool(name="med", bufs=2))
        spool = ctx.enter_context(tc.tile_pool(name="scr", bufs=3))
        smpool = ctx.enter_context(tc.tile_pool(name="sm", bufs=2))
        cpool = ctx.enter_context(tc.tile_pool(name="cst", bufs=1))
        ppool = ctx.enter_context(tc.tile_pool(name="ps", bufs=2, space="PSUM"))

        # constants
        iota_i = cpool.tile([PBLK, N_MAX], I32, tag="iota_i", name="iota_i")
        nc.gpsimd.iota(iota_i[:], pattern=[[1, N_MAX]], base=0, channel_multiplier=0)
        iota_f = cpool.tile([PBLK, N_MAX], F32, tag="iota_f", name="iota_f")
        nc.vector.tensor_copy(out=iota_f[:], in_=iota_i[:])
        ones = cpool.tile([PBLK, 1], F32, tag="ones", name="ones")
        nc.vector.memset(ones[:], 1.0)
        alpha_t = cpool.tile([PBLK, 1], F32, tag="alpha_t", name="alpha_t")
        nc.vector.memset(alpha_t[:], ALPHA)
        nbeta_t = cpool.tile([PBLK, 1], F32, tag="nbeta_t", name="nbeta_t")
        nc.vector.memset(nbeta_t[:], -BETA)

        def sm(tag, dt=F32, w=1):
            return smpool.tile([PBLK, w], dt, tag=tag, name=tag)

        blk_contribs = []
        for s in range(NBLK):
            NS = bounds[s]
            q_t = qpool.tile([PBLK, D], F32, tag="q_t", name="q_t")
            nc.sync.dma_start(out=q_t[:], in_=q[s])
            k_t = qpool.tile([PBLK, D], F32, tag="k_t", name="k_t")
            nc.sync.dma_start(out=k_t[:], in_=k[s])
            k2_t = qpool.tile([PBLK, D], F32, tag="k2_t", name="k2_t")
            nc.sync.dma_start(out=k2_t[:], in_=k2[s])
            hc_i = sm("hc_i", I32)
            nc.sync.dma_start(out=hc_i[:], in_=hc[s])
            fc_i = sm("fc_i", I32)
            nc.sync.dma_start(out=fc_i[:], in_=fc[s])
            hc_f = sm("hc_f")
            nc.vector.tensor_copy(out=hc_f[:], in_=hc_i[:])
            fc_f = sm("fc_f")
            nc.vector.tensor_copy(out=fc_f[:], in_=fc_i[:])

            dots = spool.tile([PBLK, D], F32, tag="dots", name="dots")
            adump = spool.tile([PBLK, D], F32, tag="adump", name="adump", bufs=1)

            def rowdot(in1, scale, accum):
                nc.vector.scalar_tensor_tensor(
                    out=dots[:], in0=q_t[:], scalar=scale, in1=in1,
                    op0=OP.mult, op1=OP.mult, accum_out=accum,
                )

            lpos = sm("lpos")
            rowdot(k_t[:], 1.0 / TEMP, lpos[:])
            lposnb = sm("lposnb")
            rowdot(k2_t[:], 1.0 / TEMP, lposnb[:])
            simpos = sm("simpos")
            rowdot(k_t[:], 1.0, simpos[:])

            # negative logits: lneg[b, n] = q.hn[b, n] / TEMP
            # DVE does one (128, 8, 512) multiply per chunk (q broadcast on
            # n); ScalarE reduces each negative with Copy+accum (scale=1/T).
            lneg = mpool.tile([PBLK, N_MAX], F32, tag="lneg", name="lneg")
            for c in range(NS // NCHUNK):
                h_t = hpool.tile([PBLK, NCHUNK, D], F32, tag="h_t", name="h_t")
                nc.sync.dma_start(
                    out=h_t[:], in_=hns[s][:, c * NCHUNK:(c + 1) * NCHUNK, :]
                )
                prod = spool.tile([PBLK, NCHUNK, D], F32, tag="prod", name="prod")
                nc.vector.tensor_mul(out=prod[:], in0=h_t[:],
                                     in1=_bcast_n(q_t[:], NCHUNK))
                for j in range(NCHUNK):
                    n = c * NCHUNK + j
                    nc.scalar.activation(
                        out=adump[:], in_=prod[:, j, :], func=AF.Copy,
                        scale=1.0 / TEMP, accum_out=lneg[:, n:n + 1],
                    )

            # fn dots: q.fn[b, m]; reduce on ScalarE as well
            fnd = sm("fnd", w=M_MAX)
            for c in range(M_MAX // NCHUNK):
                f_t = fpool.tile([PBLK, NCHUNK, D], F32, tag="f_t", name="f_t")
                nc.sync.dma_start(
                    out=f_t[:], in_=fn[s, :, c * NCHUNK:(c + 1) * NCHUNK, :]
                )
                prodf = spool.tile([PBLK, NCHUNK, D], F32, tag="prod", name="prodf")
                nc.vector.tensor_mul(out=prodf[:], in0=f_t[:],
                                     in1=_bcast_n(q_t[:], NCHUNK))
                for j in range(NCHUNK):
                    m = c * NCHUNK + j
                    nc.scalar.activation(
                        out=adump[:], in_=prodf[:, j, :], func=AF.Copy,
                        scale=1.0, accum_out=fnd[:, m:m + 1],
                    )

            # mask padded negatives to -1e30, then logsumexp along free axis
            mneg = mpool.tile([PBLK, N_MAX], F32, tag="mneg", name="mneg")
            nc.vector.tensor_scalar(
                out=mneg[:, :NS], in0=iota_f[:, :NS], scalar1=hc_f[:],
                scalar2=NEG, op0=OP.is_ge, op1=OP.mult,
            )
            nc.vector.tensor_add(out=lneg[:, :NS], in0=lneg[:, :NS],
                                 in1=mneg[:, :NS])
            mrow = sm("mrow")
            nc.vector.tensor_reduce(out=mrow[:], in_=lneg[:, :NS], axis=AX.X,
                                    op=OP.max)
            nmrow = sm("nmrow")
            nc.vector.tensor_scalar_mul(out=nmrow[:], in0=mrow[:], scalar1=-1.0)
            expin = mpool.tile([PBLK, N_MAX], F32, tag="expin", name="expin")
            nc.vector.tensor_scalar(
                out=expin[:, :NS], in0=lneg[:, :NS], scalar1=nmrow[:],
                scalar2=EXP_CLAMP, op0=OP.add, op1=OP.max,
            )
            expout = mpool.tile([PBLK, N_MAX], F32, tag="expout", name="expout")
            sumexp = sm("sumexp")
            nc.scalar.activation(
                out=expout[:, :NS], in_=expin[:, :NS], func=AF.Exp,
                accum_out=sumexp[:],
            )
            lse = sm("lse")
            nc.scalar.activation(out=lse[:], in_=sumexp[:], func=AF.Ln)
            nc.vector.tensor_add(out=lse[:], in0=lse[:], in1=mrow[:])

            # ce(lp) = logaddexp(lp, lse) - lp
            def ce(lp, tag):
                mm = sm("mm" + tag)
                nc.vector.tensor_max(out=mm[:], in0=lp[:], in1=lse[:])
                nmm = sm("nmm" + tag)
                nc.vector.tensor_scalar_mul(out=nmm[:], in0=mm[:], scalar1=-1.0)
                e1 = sm("e1" + tag)
                nc.vector.tensor_scalar(
                    out=e1[:], in0=lp[:], scalar1=nmm[:], scalar2=EXP_CLAMP,
                    op0=OP.add, op1=OP.max,
                )
                nc.scalar.activation(out=e1[:], in_=e1[:], func=AF.Exp)
                e2 = sm("e2" + tag)
                nc.vector.tensor_scalar(
                    out=e2[:], in0=lse[:], scalar1=nmm[:], scalar2=EXP_CLAMP,
                    op0=OP.add, op1=OP.max,
                )
                nc.scalar.activation(out=e2[:], in_=e2[:], func=AF.Exp)
                s12 = sm("s12" + tag)
                nc.vector.tensor_add(out=s12[:], in0=e1[:], in1=e2[:])
                nc.scalar.activation(out=s12[:], in_=s12[:], func=AF.Ln)
                cev = sm("ce" + tag)
                nc.vector.tensor_add(out=cev[:], in0=s12[:], in1=mm[:])
                nc.vector.tensor_sub(out=cev[:], in0=cev[:], in1=lp[:])
                return cev

            cep = ce(lpos, "p")
            cenb = ce(lposnb, "n")

            # BML term
            maskf = sm("maskf", w=M_MAX)
            nc.vector.tensor_scalar(
                out=maskf[:], in0=iota_f[:, :M_MAX], scalar1=fc_f[:],
                scalar2=None, op0=OP.is_lt,
            )
            nc.vector.tensor_mul(out=fnd[:], in0=fnd[:], in1=maskf[:])
            sfn = sm("sfn")
            nc.vector.tensor_reduce(out=sfn[:], in_=fnd[:], axis=AX.X, op=OP.add)
            den = sm("den")
            nc.vector.tensor_scalar_max(out=den[:], in0=fc_f[:], scalar1=1.0)
            rden = sm("rden")
            nc.vector.reciprocal(out=rden[:], in_=den[:])
            simfn = sm("simfn")
            nc.vector.tensor_mul(out=simfn[:], in0=sfn[:], in1=rden[:])
            delta = sm("delta")
            nc.vector.tensor_sub(out=delta[:], in0=simfn[:], in1=simpos[:])
            r1 = sm("r1")
            nc.scalar.activation(out=r1[:], in_=delta[:], func=AF.Relu,
                                 bias=alpha_t[:], scale=1.0)
            r2 = sm("r2")
            nc.scalar.activation(out=r2[:], in_=delta[:], func=AF.Relu,
                                 bias=nbeta_t[:], scale=-1.0)
            bml = sm("bml")
            nc.vector.tensor_add(out=bml[:], in0=r1[:], in1=r2[:])

            vh = sm("vh")
            nc.vector.tensor_scalar(out=vh[:], in0=hc_f[:], scalar1=0.0,
                                    scalar2=None, op0=OP.is_gt)
            vf = sm("vf")
            nc.vector.tensor_scalar(out=vf[:], in0=fc_f[:], scalar1=0.0,
                                    scalar2=None, op0=OP.is_gt)
            vb = sm("vb")
            nc.vector.tensor_mul(out=vb[:], in0=vh[:], in1=vf[:])

            contrib = smpool.tile([PBLK, 5], F32, tag="contrib", name="contrib")
            nc.vector.tensor_mul(out=contrib[:, 0:1], in0=cep[:], in1=vh[:])
            nc.vector.tensor_mul(out=contrib[:, 1:2], in0=cenb[:], in1=vh[:])
            nc.vector.tensor_mul(out=contrib[:, 2:3], in0=bml[:], in1=vb[:])
            nc.vector.tensor_copy(out=contrib[:, 3:4], in_=vh[:])
            nc.vector.tensor_copy(out=contrib[:, 4:5], in_=vb[:])
            blk_contribs.append(contrib)

        tot = blk_contribs[0]
        nc.vector.tensor_add(out=tot[:], in0=tot[:], in1=blk_contribs[1][:])

        ps = ppool.tile([5, 1], F32, tag="ps5", name="ps5")
        nc.tensor.matmul(ps[:], lhsT=tot[:], rhs=ones[:], start=True, stop=True)
        res = smpool.tile([5, 1], F32, tag="res", name="res")
        nc.scalar.copy(out=res[:], in_=ps[:])
        nc.sync.dma_start(out=out[:], in_=res[:])


def _build(bounds):
    N0, N1 = bounds
    nc = bacc.Bacc("TRN2", target_bir_lowering=False, debug=False)
    q = nc.dram_tensor("q", [NBLK, PBLK, D], F32, kind="ExternalInput")
    k = nc.dram_tensor("k", [NBLK, PBLK, D], F32, kind="ExternalInput")
    k2 = nc.dram_tensor("k2", [NBLK, PBLK, D], F32, kind="ExternalInput")
    hn0 = nc.dram_tensor("hn0", [PBLK, N0, D], F32, kind="ExternalInput")
    hn1 = nc.dram_tensor("hn1", [PBLK, N1, D], F32, kind="ExternalInput")
    fn = nc.dram_tensor("fn", [NBLK, PBLK, M_MAX, D], F32, kind="ExternalInput")
    hc = nc.dram_tensor("hn_counts", [NBLK, PBLK, 1], I32, kind="ExternalInput")
    fc = nc.dram_tensor("fn_counts", [NBLK, PBLK, 1], I32, kind="ExternalInput")
    out = nc.dram_tensor("out", [5, 1], F32, kind="ExternalOutput")
    with tile.TileContext(nc) as tc:
        _emit(tc, bounds, q, k, k2, (hn0, hn1), fn, hc, fc, out)
    nc.compile()
    return nc


_NC_CACHE = {}


def _get_nc(bounds):
    if bounds not in _NC_CACHE:
        _NC_CACHE[bounds] = _build(bounds)
    return _NC_CACHE[bounds]


def _round8(x):
    return max(8, int(-(-int(x) // 8) * 8))


def plan(hn_counts):
    """Global count-sorted block schedule: returns (order, (N0, N1))."""
    order = np.argsort(-hn_counts, kind="stable")
    blocks = order.reshape(2 * N_CORES, PBLK)
    c = np.asarray(hn_counts)
    n0 = _round8(c[blocks[0:N_CORES]].max())
    n1 = _round8(c[blocks[N_CORES:]].max())
    return blocks, (min(n0, N_MAX), min(n1, N_MAX))


def make_in_maps(q, k, k2, hn, fn, hn_counts, fn_counts):
    q = np.asarray(q, np.float32)
    k = np.asarray(k, np.float32)
    k2 = np.asarray(k2, np.float32)
    hn = np.asarray(hn, np.float32)
    fn = np.asarray(fn, np.float32)
    hn_counts = np.asarray(hn_counts, np.int32)
    fn_counts = np.asarray(fn_counts, np.int32)
    blocks, (n0, n1) = plan(hn_counts)
    hn_v0 = hn[:, :n0, :]   # views, no copy
    hn_v1 = hn[:, :n1, :]
    in_maps = []
    for c in range(N_CORES):
        i0, i1 = blocks[c], blocks[N_CORES + c]
        both = np.stack([i0, i1])
        in_maps.append({
            "q": q[both],
            "k": k[both],
            "k2": k2[both],
            "hn0": hn_v0[i0],
            "hn1": hn_v1[i1],
            "fn": fn[both],
            "hn_counts": hn_counts[both][..., None],
            "fn_counts": fn_counts[both][..., None],
        })
    return in_maps, (n0, n1)


def combine_partials(results):
    parts = np.stack([np.asarray(r["out"], np.float64).reshape(5) for r in results])
    cl_s, clnb_s, bml_s, nv, nb = parts.sum(axis=0)
    n_valid = max(nv, 1.0)
    cl = cl_s / n_valid
    clnb = clnb_s / n_valid
    bml_mean = (bml_s / nb) if nb > 0 else 0.0
    lbml = LAMBDA_BML * bml_mean
    tot = cl + clnb + lbml
    return np.array([tot, cl, lbml, clnb], np.float32)


def run_spmd(in_maps, bounds, **kwargs):
    nc = _get_nc(bounds)
    return bass_utils.run_bass_kernel_spmd(
        nc, in_maps, core_ids=list(range(N_CORES)), **kwargs
    )


def kernel(q, k, k2, hn, fn, hn_counts, fn_counts):
    in_maps, bounds = make_in_maps(q, k, k2, hn, fn, hn_counts, fn_counts)
    res = run_spmd(in_maps, bounds)
    return combine_partials(res.results)


# revision 11
# speedup vs baseline: 1.5245x; 1.2765x over previous
"""ClusterLoss Bass/Tile kernel for Trainium2 (8 NeuronCores, data parallel).

Strategy
--------
Pure data parallelism over B=2048 with a count-aware schedule: samples are
globally sorted by hn_count and dealt into 16 blocks of 128; every core gets
one "big" block (slot 0, padded negative bound N0) and one "small" block
(slot 1, bound N1), so a single SPMD program with static loop bounds fits all
cores while skipping most padded negatives.  The four losses are sums over
samples, so no output unpermutation is needed.

Per block, samples sit on the 128 partitions with the feature dim d on the
free axis.  The einsum('bd,bnd->bn') runs as one fused DVE tensor_tensor
multiply per 8-negative chunk (q broadcast via a stride-0 access pattern)
with the per-negative free-axis reductions on the otherwise idle Scalar
engine (activation Copy with accum_out, folding the 1/TEMP scale).  Masked
logsumexp / cross-entropy / BML terms are small per-partition vector ops.
Each core emits 5 partial sums reduced over partitions with a ones-vector
matmul; the final scalar combine runs on host.

The program is JIT-specialized to (N0, N1) derived from the counts at call
time and cached, so repeated calls with the same raggedness profile reuse
the compiled NEFF.
"""

from contextlib import ExitStack

import numpy as np

import concourse.bass as bass
import concourse.bacc as bacc
import concourse.tile as tile
from concourse import mybir
from concourse import bass_utils

N_CORES = 8
B, D, N_MAX, M_MAX = 2048, 512, 256, 32
B_LOC = B // N_CORES          # 256 samples per core
PBLK = 128                    # partition block
NBLK = B_LOC // PBLK          # 2 slots per core
NCHUNK = 8                    # negatives per hn DMA tile / DVE multiply

TEMP, ALPHA, BETA, LAMBDA_BML = 0.07, 0.4, 0.2, 0.2
NEG = -1e30
EXP_CLAMP = -87.0             # exp(-87) underflows f32; avoids LUT extremes

F32 = mybir.dt.float32
I32 = mybir.dt.int32
AF = mybir.ActivationFunctionType
OP = mybir.AluOpType
AX = mybir.AxisListType


def _bcast_n(ap, n):
    """(128, D) AP viewed as (128, n, D) with stride-0 broadcast on n."""
    return bass.AP(tensor=ap.tensor, offset=ap.offset,
                   ap=[ap.ap[0], [0, n], ap.ap[1]])


def _emit(tc, bounds, q, k, k2, hns, fn, hc, fc, out):
    nc = tc.nc
    with ExitStack() as ctx:
        hpool = ctx.enter_context(tc.tile_pool(name="hnp", bufs=4))
        fpool = ctx.enter_context(tc.tile_pool(name="fnp", bufs=2))
        qpool = ctx.enter_context(tc.tile_pool(name="qkp", bufs=2))
        mpool = ctx.enter_context(tc.tile_pool(name="med", bufs=2))
        spool = ctx.enter_context(tc.tile_pool(name="scr", bufs=3))
        smpool = ctx.enter_context(tc.tile_pool(name="sm", bufs=2))
        cpool = ctx.enter_context(tc.tile_pool(name="cst", bufs=1))
        ppool = ctx.enter_context(tc.tile_pool(name="ps", bufs=2, space="PSUM"))

        # constants
        iota_i = cpool.tile([PBLK, N_MAX], I32, tag="iota_i", name="iota_i")
        nc.gpsimd.iota(iota_i[:], pattern=[[1, N_MAX]], base=0, channel_multiplier=0)
        iota_f = cpool.tile([PBLK, N_MAX], F32, tag="iota_f", name="iota_f")
        nc.vector.tensor_copy(out=iota_f[:], in_=iota_i[:])
        ones = cpool.tile([PBLK, 1], F32, tag="ones", name="ones")
        nc.vector.memset(ones[:], 1.0)
        alpha_t = cpool.tile([PBLK, 1], F32, tag="alpha_t", name="alpha_t")
        nc.vector.memset(alpha_t[:], ALPHA)
        nbeta_t = cpool.tile([PBLK, 1], F32, tag="nbeta_t", name="nbeta_t")
        nc.vector.memset(nbeta_t[:], -BETA)

        def sm(tag, dt=F32, w=1):
            return smpool.tile([PBLK, w], dt, tag=tag, name=tag)

        blk_contribs = []
        for s in range(NBLK):
            NS = bounds[s]
            q_t = qpool.tile([PBLK, D], F32, tag="q_t", name="q_t")
            nc.sync.dma_start(out=q_t[:], in_=q[s])
            k_t = qpool.tile([PBLK, D], F32, tag="k_t", name="k_t")
            nc.sync.dma_start(out=k_t[:], in_=k[s])
            k2_t = qpool.tile([PBLK, D], F32, tag="k2_t", name="k2_t")
            nc.sync.dma_start(out=k2_t[:], in_=k2[s])
            hc_i = sm("hc_i", I32)
            nc.sync.dma_start(out=hc_i[:], in_=hc[s])
            fc_i = sm("fc_i", I32)
            nc.sync.dma_start(out=fc_i[:], in_=fc[s])
            hc_f = sm("hc_f")
            nc.vector.tensor_copy(out=hc_f[:], in_=hc_i[:])
            fc_f = sm("fc_f")
            nc.vector.tensor_copy(out=fc_f[:], in_=fc_i[:])

            dots = spool.tile([PBLK, D], F32, tag="dots", name="dots")
            adump = spool.tile([PBLK, D], F32, tag="adump", name="adump", bufs=1)

            def rowdot(in1, scale, accum):
                nc.vector.scalar_tensor_tensor(
                    out=dots[:], in0=q_t[:], scalar=scale, in1=in1,
                    op0=OP.mult, op1=OP.mult, accum_out=accum,
                )

            lpos = sm("lpos")
            rowdot(k_t[:], 1.0 / TEMP, lpos[:])
            lposnb = sm("lposnb")
            rowdot(k2_t[:], 1.0 / TEMP, lposnb[:])
            simpos = sm("simpos")
            rowdot(k_t[:], 1.0, simpos[:])

            # negative logits: lneg[b, n] = q.hn[b, n] / TEMP
            # DVE does one (128, 8, 512) multiply per chunk (q broadcast on
            # n, 1/TEMP folded in); per-negative free-axis reductions are
            # split R_DVE:NCHUNK-R_DVE between DVE (one multi-n
            # tensor_reduce) and the otherwise idle ScalarE (Copy+accum) so
            # both engines run near their line rate.
            R_DVE = 2
            lneg = mpool.tile([PBLK, N_MAX], F32, tag="lneg", name="lneg")
            for c in range(NS // NCHUNK):
                n0 = c * NCHUNK
                h_t = hpool.tile([PBLK, NCHUNK, D], F32, tag="h_t", name="h_t")
                nc.sync.dma_start(
                    out=h_t[:], in_=hns[s][:, n0:n0 + NCHUNK, :]
                )
                prod = spool.tile([PBLK, NCHUNK, D], F32, tag="prod", name="prod")
                nc.vector.scalar_tensor_tensor(
                    out=prod[:], in0=h_t[:], scalar=1.0 / TEMP,
                    in1=_bcast_n(q_t[:], NCHUNK), op0=OP.mult, op1=OP.mult,
                )
                nc.vector.tensor_reduce(
                    out=lneg[:, n0:n0 + R_DVE], in_=prod[:, :R_DVE, :],
                    axis=AX.X, op=OP.add,
                )
                for j in range(R_DVE, NCHUNK):
                    nc.scalar.activation(
                        out=adump[:], in_=prod[:, j, :], func=AF.Copy,
                        scale=1.0, accum_out=lneg[:, n0 + j:n0 + j + 1],
                    )

            # fn dots: q.fn[b, m]; same split, no TEMP scale
            fnd = sm("fnd", w=M_MAX)
            for c in range(M_MAX // NCHUNK):
                m0 = c * NCHUNK
                f_t = fpool.tile([PBLK, NCHUNK, D], F32, tag="f_t", name="f_t")
                nc.sync.dma_start(
                    out=f_t[:], in_=fn[s, :, m0:m0 + NCHUNK, :]
                )
                prodf = spool.tile([PBLK, NCHUNK, D], F32, tag="prod", name="prodf")
                nc.vector.tensor_mul(out=prodf[:], in0=f_t[:],
                                     in1=_bcast_n(q_t[:], NCHUNK))
                nc.vector.tensor_reduce(
                    out=fnd[:, m0:m0 + R_DVE], in_=prodf[:, :R_DVE, :],
                    axis=AX.X, op=OP.add,
                )
                for j in range(R_DVE, NCHUNK):
                    nc.scalar.activation(
                        out=adump[:], in_=prodf[:, j, :], func=AF.Copy,
                        scale=1.0, accum_out=fnd[:, m0 + j:m0 + j + 1],
                    )

            # mask padded negatives to -1e30, then logsumexp along free axis
            mneg = mpool.tile([PBLK, N_MAX], F32, tag="mneg", name="mneg")
            nc.vector.tensor_scalar(
                out=mneg[:, :NS], in0=iota_f[:, :NS], scalar1=hc_f[:],
                scalar2=NEG, op0=OP.is_ge, op1=OP.mult,
            )
            nc.vector.tensor_add(out=lneg[:, :NS], in0=lneg[:, :NS],
                                 in1=mneg[:, :NS])
            mrow = sm("mrow")
            nc.vector.tensor_reduce(out=mrow[:], in_=lneg[:, :NS], axis=AX.X,
                                    op=OP.max)
            nmrow = sm("nmrow")
            nc.vector.tensor_scalar_mul(out=nmrow[:], in0=mrow[:], scalar1=-1.0)
            expin = mpool.tile([PBLK, N_MAX], F32, tag="expin", name="expin")
            nc.vector.tensor_scalar(
                out=expin[:, :NS], in0=lneg[:, :NS], scalar1=nmrow[:],
                scalar2=EXP_CLAMP, op0=OP.add, op1=OP.max,
            )
            expout = mpool.tile([PBLK, N_MAX], F32, tag="expout", name="expout")
            sumexp = sm("sumexp")
            nc.scalar.activation(
                out=expout[:, :NS], in_=expin[:, :NS], func=AF.Exp,
                accum_out=sumexp[:],
            )
            lse = sm("lse")
            nc.scalar.activation(out=lse[:], in_=sumexp[:], func=AF.Ln)
            nc.vector.tensor_add(out=lse[:], in0=lse[:], in1=mrow[:])

            # ce(lp) = logaddexp(lp, lse) - lp
            def ce(lp, tag):
                mm = sm("mm" + tag)
                nc.vector.tensor_max(out=mm[:], in0=lp[:], in1=lse[:])
                nmm = sm("nmm" + tag)
                nc.vector.tensor_scalar_mul(out=nmm[:], in0=mm[:], scalar1=-1.0)
                e1 = sm("e1" + tag)
                nc.vector.tensor_scalar(
                    out=e1[:], in0=lp[:], scalar1=nmm[:], scalar2=EXP_CLAMP,
                    op0=OP.add, op1=OP.max,
                )
                nc.scalar.activation(out=e1[:], in_=e1[:], func=AF.Exp)
                e2 = sm("e2" + tag)
                nc.vector.tensor_scalar(
                    out=e2[:], in0=lse[:], scalar1=nmm[:], scalar2=EXP_CLAMP,
                    op0=OP.add, op1=OP.max,
                )
                nc.scalar.activation(out=e2[:], in_=e2[:], func=AF.Exp)
                s12 = sm("s12" + tag)
                nc.vector.tensor_add(out=s12[:], in0=e1[:], in1=e2[:])
                nc.scalar.activation(out=s12[:], in_=s12[:], func=AF.Ln)
                cev = sm("ce" + tag)
                nc.vector.tensor_add(out=cev[:], in0=s12[:], in1=mm[:])
                nc.vector.tensor_sub(out=cev[:], in0=cev[:], in1=lp[:])
                return cev

            cep = ce(lpos, "p")
            cenb = ce(lposnb, "n")

            # BML term
            maskf = sm("maskf", w=M_MAX)
            nc.vector.tensor_scalar(
                out=maskf[:], in0=iota_f[:, :M_MAX], scalar1=fc_f[:],
                scalar2=None, op0=OP.is_lt,
            )
            nc.vector.tensor_mul(out=fnd[:], in0=fnd[:], in1=maskf[:])
            sfn = sm("sfn")
            nc.vector.tensor_reduce(out=sfn[:], in_=fnd[:], axis=AX.X, op=OP.add)
            den = sm("den")
            nc.vector.tensor_scalar_max(out=den[:], in0=fc_f[:], scalar1=1.0)
            rden = sm("rden")
            nc.vector.reciprocal(out=rden[:], in_=den[:])
            simfn = sm("simfn")
            nc.vector.tensor_mul(out=simfn[:], in0=sfn[:], in1=rden[:])
            delta = sm("delta")
            nc.vector.tensor_sub(out=delta[:], in0=simfn[:], in1=simpos[:])
            r1 = sm("r1")
            nc.scalar.activation(out=r1[:], in_=delta[:], func=AF.Relu,
                                 bias=alpha_t[:], scale=1.0)
            r2 = sm("r2")
            nc.scalar.activation(out=r2[:], in_=delta[:], func=AF.Relu,
                                 bias=nbeta_t[:], scale=-1.0)
            bml = sm("bml")
            nc.vector.tensor_add(out=bml[:], in0=r1[:], in1=r2[:])

            vh = sm("vh")
            nc.vector.tensor_scalar(out=vh[:], in0=hc_f[:], scalar1=0.0,
                                    scalar2=None, op0=OP.is_gt)
            vf = sm("vf")
            nc.vector.tensor_scalar(out=vf[:], in0=fc_f[:], scalar1=0.0,
                                    scalar2=None, op0=OP.is_gt)
            vb = sm("vb")
            nc.vector.tensor_mul(out=vb[:], in0=vh[:], in1=vf[:])

            contrib = smpool.tile([PBLK, 5], F32, tag="contrib", name="contrib")
            nc.vector.tensor_mul(out=contrib[:, 0:1], in0=cep[:], in1=vh[:])
            nc.vector.tensor_mul(out=contrib[:, 1:2], in0=cenb[:], in1=vh[:])
            nc.vector.tensor_mul(out=contrib[:, 2:3], in0=bml[:], in1=vb[:])
            nc.vector.tensor_copy(out=contrib[:, 3:4], in_=vh[:])
            nc.vector.tensor_copy(out=contrib[:, 4:5], in_=vb[:])
            blk_contribs.append(contrib)

        tot = blk_contribs[0]
        nc.vector.tensor_add(out=tot[:], in0=tot[:], in1=blk_contribs[1][:])

        ps = ppool.tile([5, 1], F32, tag="ps5", name="ps5")
        nc.tensor.matmul(ps[:], lhsT=tot[:], rhs=ones[:], start=True, stop=True)
        res = smpool.tile([5, 1], F32, tag="res", name="res")
        nc.scalar.copy(out=res[:], in_=ps[:])
        nc.sync.dma_start(out=out[:], in_=res[:])


def _build(bounds):
    N0, N1 = bounds
    nc = bacc.Bacc("TRN2", target_bir_lowering=False, debug=False)
    q = nc.dram_tensor("q", [NBLK, PBLK, D], F32, kind="ExternalInput")
    k = nc.dram_tensor("k", [NBLK, PBLK, D], F32, kind="ExternalInput")
    k2 = nc.dram_tensor("k2", [NBLK, PBLK, D], F32, kind="ExternalInput")
    hn0 = nc.dram_tensor("hn0", [PBLK, N0, D], F32, kind="ExternalInput")
    hn1 = nc.dram_tensor("hn1", [PBLK, N1, D], F32, kind="ExternalInput")
    fn = nc.dram_tensor("fn", [NBLK, PBLK, M_MAX, D], F32, kind="ExternalInput")
    hc = nc.dram_tensor("hn_counts", [NBLK, PBLK, 1], I32, kind="ExternalInput")
    fc = nc.dram_tensor("fn_counts", [NBLK, PBLK, 1], I32, kind="ExternalInput")
    out = nc.dram_tensor("out", [5, 1], F32, kind="ExternalOutput")
    with tile.TileContext(nc) as tc:
        _emit(tc, bounds, q, k, k2, (hn0, hn1), fn, hc, fc, out)
    nc.compile()
    return nc


_NC_CACHE = {}


def _get_nc(bounds):
    if bounds not in _NC_CACHE:
        _NC_CACHE[bounds] = _build(bounds)
    return _NC_CACHE[bounds]


def _round8(x):
    return max(8, int(-(-int(x) // 8) * 8))


def plan(hn_counts):
    """Global count-sorted block schedule: returns (order, (N0, N1))."""
    order = np.argsort(-hn_counts, kind="stable")
    blocks = order.reshape(2 * N_CORES, PBLK)
    c = np.asarray(hn_counts)
    n0 = _round8(c[blocks[0:N_CORES]].max())
    n1 = _round8(c[blocks[N_CORES:]].max())
    return blocks, (min(n0, N_MAX), min(n1, N_MAX))


def make_in_maps(q, k, k2, hn, fn, hn_counts, fn_counts):
    q = np.asarray(q, np.float32)
    k = np.asarray(k, np.float32)
    k2 = np.asarray(k2, np.float32)
    hn = np.asarray(hn, np.float32)
    fn = np.asarray(fn, np.float32)
    hn_counts = np.asarray(hn_counts, np.int32)
    fn_counts = np.asarray(fn_counts, np.int32)
    blocks, (n0, n1) = plan(hn_counts)
    hn_v0 = hn[:, :n0, :]   # views, no copy
    hn_v1 = hn[:, :n1, :]
    in_maps = []
    for c in range(N_CORES):
        i0, i1 = blocks[c], blocks[N_CORES + c]
        both = np.stack([i0, i1])
        in_maps.append({
            "q": q[both],
            "k": k[both],
            "k2": k2[both],
            "hn0": hn_v0[i0],
            "hn1": hn_v1[i1],
            "fn": fn[both],
            "hn_counts": hn_counts[both][..., None],
            "fn_counts": fn_counts[both][..., None],
        })
    return in_maps, (n0, n1)


def combine_partials(results):
    parts = np.stack([np.asarray(r["out"], np.float64).reshape(5) for r in results])
    cl_s, clnb_s, bml_s, nv, nb = parts.sum(axis=0)
    n_valid = max(nv, 1.0)
    cl = cl_s / n_valid
    clnb = clnb_s / n_valid
    bml_mean = (bml_s / nb) if nb > 0 else 0.0
    lbml = LAMBDA_BML * bml_mean
    tot = cl + clnb + lbml
    return np.array([tot, cl, lbml, clnb], np.float32)


def run_spmd(in_maps, bounds, **kwargs):
    nc = _get_nc(bounds)
    return bass_utils.run_bass_kernel_spmd(
        nc, in_maps, core_ids=list(range(N_CORES)), **kwargs
    )


def kernel(q, k, k2, hn, fn, hn_counts, fn_counts):
    in_maps, bounds = make_in_maps(q, k, k2, hn, fn, hn_counts, fn_counts)
    res = run_spmd(in_maps, bounds)
    return combine_partials(res.results)


# revision 12
# speedup vs baseline: 1.5630x; 1.0252x over previous
"""ClusterLoss Bass/Tile kernel for Trainium2 (8 NeuronCores, data parallel).

Strategy
--------
Pure data parallelism over B=2048 with a count-aware schedule: samples are
globally sorted by hn_count and dealt into 16 blocks of 128; every core gets
one "big" block (slot 0, padded negative bound N0) and one "small" block
(slot 1, bound N1), so a single SPMD program with static loop bounds fits all
cores while skipping most padded negatives.  The four losses are sums over
samples, so no output unpermutation is needed.

Per block, samples sit on the 128 partitions with the feature dim d on the
free axis.  The einsum('bd,bnd->bn') runs as one fused DVE tensor_tensor
multiply per 8-negative chunk (q broadcast via a stride-0 access pattern)
with the per-negative free-axis reductions on the otherwise idle Scalar
engine (activation Copy with accum_out, folding the 1/TEMP scale).  Masked
logsumexp / cross-entropy / BML terms are small per-partition vector ops.
Each core emits 5 partial sums reduced over partitions with a ones-vector
matmul; the final scalar combine runs on host.

The program is JIT-specialized to (N0, N1) derived from the counts at call
time and cached, so repeated calls with the same raggedness profile reuse
the compiled NEFF.
"""

from contextlib import ExitStack

import numpy as np

import concourse.bass as bass
import concourse.bacc as bacc
import concourse.tile as tile
from concourse import mybir
from concourse import bass_utils

N_CORES = 8
B, D, N_MAX, M_MAX = 2048, 512, 256, 32
B_LOC = B // N_CORES          # 256 samples per core
PBLK = 128                    # partition block
NBLK = B_LOC // PBLK          # 2 slots per core
NCHUNK = 8                    # negatives per hn DMA tile / DVE multiply

TEMP, ALPHA, BETA, LAMBDA_BML = 0.07, 0.4, 0.2, 0.2
NEG = -1e30
EXP_CLAMP = -87.0             # exp(-87) underflows f32; avoids LUT extremes

F32 = mybir.dt.float32
F16 = mybir.dt.float16
I32 = mybir.dt.int32
AF = mybir.ActivationFunctionType
OP = mybir.AluOpType
AX = mybir.AxisListType


def _bcast_n(ap, n):
    """(128, D) AP viewed as (128, n, D) with stride-0 broadcast on n."""
    return bass.AP(tensor=ap.tensor, offset=ap.offset,
                   ap=[ap.ap[0], [0, n], ap.ap[1]])


def _emit(tc, bounds, q, q16, k, k2, hns, fn, hc, fc, out):
    nc = tc.nc
    with ExitStack() as ctx:
        hpool = ctx.enter_context(tc.tile_pool(name="hnp", bufs=6))
        fpool = ctx.enter_context(tc.tile_pool(name="fnp", bufs=2))
        qpool = ctx.enter_context(tc.tile_pool(name="qkp", bufs=2))
        mpool = ctx.enter_context(tc.tile_pool(name="med", bufs=2))
        spool = ctx.enter_context(tc.tile_pool(name="scr", bufs=3))
        smpool = ctx.enter_context(tc.tile_pool(name="sm", bufs=2))
        cpool = ctx.enter_context(tc.tile_pool(name="cst", bufs=1))
        ppool = ctx.enter_context(tc.tile_pool(name="ps", bufs=2, space="PSUM"))

        # constants
        iota_i = cpool.tile([PBLK, N_MAX], I32, tag="iota_i", name="iota_i")
        nc.gpsimd.iota(iota_i[:], pattern=[[1, N_MAX]], base=0, channel_multiplier=0)
        iota_f = cpool.tile([PBLK, N_MAX], F32, tag="iota_f", name="iota_f")
        nc.vector.tensor_copy(out=iota_f[:], in_=iota_i[:])
        ones = cpool.tile([PBLK, 1], F32, tag="ones", name="ones")
        nc.vector.memset(ones[:], 1.0)
        alpha_t = cpool.tile([PBLK, 1], F32, tag="alpha_t", name="alpha_t")
        nc.vector.memset(alpha_t[:], ALPHA)
        nbeta_t = cpool.tile([PBLK, 1], F32, tag="nbeta_t", name="nbeta_t")
        nc.vector.memset(nbeta_t[:], -BETA)

        def sm(tag, dt=F32, w=1):
            return smpool.tile([PBLK, w], dt, tag=tag, name=tag)

        blk_contribs = []
        for s in range(NBLK):
            NS = bounds[s]
            q_t = qpool.tile([PBLK, D], F32, tag="q_t", name="q_t")
            nc.sync.dma_start(out=q_t[:], in_=q[s])
            k_t = qpool.tile([PBLK, D], F32, tag="k_t", name="k_t")
            nc.sync.dma_start(out=k_t[:], in_=k[s])
            k2_t = qpool.tile([PBLK, D], F32, tag="k2_t", name="k2_t")
            nc.sync.dma_start(out=k2_t[:], in_=k2[s])
            q16_t = qpool.tile([PBLK, D], F16, tag="q16_t", name="q16_t")
            nc.sync.dma_start(out=q16_t[:], in_=q16[s])
            hc_i = sm("hc_i", I32)
            nc.sync.dma_start(out=hc_i[:], in_=hc[s])
            fc_i = sm("fc_i", I32)
            nc.sync.dma_start(out=fc_i[:], in_=fc[s])
            hc_f = sm("hc_f")
            nc.vector.tensor_copy(out=hc_f[:], in_=hc_i[:])
            fc_f = sm("fc_f")
            nc.vector.tensor_copy(out=fc_f[:], in_=fc_i[:])

            dots = spool.tile([PBLK, D], F32, tag="dots", name="dots")
            adump = spool.tile([PBLK, D], F16, tag="adump", name="adump", bufs=1)

            def rowdot(in1, scale, accum):
                nc.vector.scalar_tensor_tensor(
                    out=dots[:], in0=q_t[:], scalar=scale, in1=in1,
                    op0=OP.mult, op1=OP.mult, accum_out=accum,
                )

            lpos = sm("lpos")
            rowdot(k_t[:], 1.0 / TEMP, lpos[:])
            lposnb = sm("lposnb")
            rowdot(k2_t[:], 1.0 / TEMP, lposnb[:])
            simpos = sm("simpos")
            rowdot(k_t[:], 1.0, simpos[:])

            # negative logits: lneg[b, n] = q.hn[b, n] / TEMP
            # DVE does one (128, 8, 512) multiply per chunk (q broadcast on
            # n, 1/TEMP folded in); per-negative free-axis reductions are
            # split R_DVE:NCHUNK-R_DVE between DVE (one multi-n
            # tensor_reduce) and the otherwise idle ScalarE (Copy+accum) so
            # both engines run near their line rate.
            lneg = mpool.tile([PBLK, N_MAX], F32, tag="lneg", name="lneg")
            for c in range(NS // NCHUNK):
                r_dve = 3 if c % 2 else 2
                n0 = c * NCHUNK
                h_t = hpool.tile([PBLK, NCHUNK, D], F16, tag="h_t", name="h_t")
                nc.sync.dma_start(
                    out=h_t[:], in_=hns[s][:, n0:n0 + NCHUNK, :]
                )
                prod = spool.tile([PBLK, NCHUNK, D], F16, tag="prod", name="prod")
                nc.vector.scalar_tensor_tensor(
                    out=prod[:], in0=h_t[:], scalar=1.0 / TEMP,
                    in1=_bcast_n(q16_t[:], NCHUNK), op0=OP.mult, op1=OP.mult,
                )
                nc.vector.tensor_reduce(
                    out=lneg[:, n0:n0 + r_dve], in_=prod[:, :r_dve, :],
                    axis=AX.X, op=OP.add,
                )
                for j in range(r_dve, NCHUNK):
                    nc.scalar.activation(
                        out=adump[:], in_=prod[:, j, :], func=AF.Copy,
                        scale=1.0, accum_out=lneg[:, n0 + j:n0 + j + 1],
                    )

            # fn dots: q.fn[b, m]; same split, no TEMP scale
            fnd = sm("fnd", w=M_MAX)
            for c in range(M_MAX // NCHUNK):
                r_dve = 3 if c % 2 else 2
                m0 = c * NCHUNK
                f_t = fpool.tile([PBLK, NCHUNK, D], F16, tag="f_t", name="f_t")
                nc.sync.dma_start(
                    out=f_t[:], in_=fn[s, :, m0:m0 + NCHUNK, :]
                )
                prodf = spool.tile([PBLK, NCHUNK, D], F16, tag="prod", name="prodf")
                nc.vector.tensor_mul(out=prodf[:], in0=f_t[:],
                                     in1=_bcast_n(q16_t[:], NCHUNK))
                nc.vector.tensor_reduce(
                    out=fnd[:, m0:m0 + r_dve], in_=prodf[:, :r_dve, :],
                    axis=AX.X, op=OP.add,
                )
                for j in range(r_dve, NCHUNK):
                    nc.scalar.activation(
                        out=adump[:], in_=prodf[:, j, :], func=AF.Copy,
                        scale=1.0, accum_out=fnd[:, m0 + j:m0 + j + 1],
                    )

            # mask padded negatives to -1e30, then logsumexp along free axis
            mneg = mpool.tile([PBLK, N_MAX], F32, tag="mneg", name="mneg")
            nc.vector.tensor_scalar(
                out=mneg[:, :NS], in0=iota_f[:, :NS], scalar1=hc_f[:],
                scalar2=NEG, op0=OP.is_ge, op1=OP.mult,
            )
            nc.vector.tensor_add(out=lneg[:, :NS], in0=lneg[:, :NS],
                                 in1=mneg[:, :NS])
            mrow = sm("mrow")
            nc.vector.tensor_reduce(out=mrow[:], in_=lneg[:, :NS], axis=AX.X,
                                    op=OP.max)
            nmrow = sm("nmrow")
            nc.vector.tensor_scalar_mul(out=nmrow[:], in0=mrow[:], scalar1=-1.0)
            expin = mpool.tile([PBLK, N_MAX], F32, tag="expin", name="expin")
            nc.vector.tensor_scalar(
                out=expin[:, :NS], in0=lneg[:, :NS], scalar1=nmrow[:],
                scalar2=EXP_CLAMP, op0=OP.add, op1=OP.max,
            )
            expout = mpool.tile([PBLK, N_MAX], F32, tag="expout", name="expout")
            sumexp = sm("sumexp")
            nc.scalar.activation(
                out=expout[:, :NS], in_=expin[:, :NS], func=AF.Exp,
                accum_out=sumexp[:],
            )
            lse = sm("lse")
            nc.scalar.activation(out=lse[:], in_=sumexp[:], func=AF.Ln)
            nc.vector.tensor_add(out=lse[:], in0=lse[:], in1=mrow[:])

            # ce(lp) = logaddexp(lp, lse) - lp
            def ce(lp, tag):
                mm = sm("mm" + tag)
                nc.vector.tensor_max(out=mm[:], in0=lp[:], in1=lse[:])
                nmm = sm("nmm" + tag)
                nc.vector.tensor_scalar_mul(out=nmm[:], in0=mm[:], scalar1=-1.0)
                e1 = sm("e1" + tag)
                nc.vector.tensor_scalar(
                    out=e1[:], in0=lp[:], scalar1=nmm[:], scalar2=EXP_CLAMP,
                    op0=OP.add, op1=OP.max,
                )
                nc.scalar.activation(out=e1[:], in_=e1[:], func=AF.Exp)
                e2 = sm("e2" + tag)
                nc.vector.tensor_scalar(
                    out=e2[:], in0=lse[:], scalar1=nmm[:], scalar2=EXP_CLAMP,
                    op0=OP.add, op1=OP.max,
                )
                nc.scalar.activation(out=e2[:], in_=e2[:], func=AF.Exp)
                s12 = sm("s12" + tag)
                nc.vector.tensor_add(out=s12[:], in0=e1[:], in1=e2[:])
                nc.scalar.activation(out=s12[:], in_=s12[:], func=AF.Ln)
                cev = sm("ce" + tag)
                nc.vector.tensor_add(out=cev[:], in0=s12[:], in1=mm[:])
                nc.vector.tensor_sub(out=cev[:], in0=cev[:], in1=lp[:])
                return cev

            cep = ce(lpos, "p")
            cenb = ce(lposnb, "n")

            # BML term
            maskf = sm("maskf", w=M_MAX)
            nc.vector.tensor_scalar(
                out=maskf[:], in0=iota_f[:, :M_MAX], scalar1=fc_f[:],
                scalar2=None, op0=OP.is_lt,
            )
            nc.vector.tensor_mul(out=fnd[:], in0=fnd[:], in1=maskf[:])
            sfn = sm("sfn")
            nc.vector.tensor_reduce(out=sfn[:], in_=fnd[:], axis=AX.X, op=OP.add)
            den = sm("den")
            nc.vector.tensor_scalar_max(out=den[:], in0=fc_f[:], scalar1=1.0)
            rden = sm("rden")
            nc.vector.reciprocal(out=rden[:], in_=den[:])
            simfn = sm("simfn")
            nc.vector.tensor_mul(out=simfn[:], in0=sfn[:], in1=rden[:])
            delta = sm("delta")
            nc.vector.tensor_sub(out=delta[:], in0=simfn[:], in1=simpos[:])
            r1 = sm("r1")
            nc.scalar.activation(out=r1[:], in_=delta[:], func=AF.Relu,
                                 bias=alpha_t[:], scale=1.0)
            r2 = sm("r2")
            nc.scalar.activation(out=r2[:], in_=delta[:], func=AF.Relu,
                                 bias=nbeta_t[:], scale=-1.0)
            bml = sm("bml")
            nc.vector.tensor_add(out=bml[:], in0=r1[:], in1=r2[:])

            vh = sm("vh")
            nc.vector.tensor_scalar(out=vh[:], in0=hc_f[:], scalar1=0.0,
                                    scalar2=None, op0=OP.is_gt)
            vf = sm("vf")
            nc.vector.tensor_scalar(out=vf[:], in0=fc_f[:], scalar1=0.0,
                                    scalar2=None, op0=OP.is_gt)
            vb = sm("vb")
            nc.vector.tensor_mul(out=vb[:], in0=vh[:], in1=vf[:])

            contrib = smpool.tile([PBLK, 5], F32, tag="contrib", name="contrib")
            nc.vector.tensor_mul(out=contrib[:, 0:1], in0=cep[:], in1=vh[:])
            nc.vector.tensor_mul(out=contrib[:, 1:2], in0=cenb[:], in1=vh[:])
            nc.vector.tensor_mul(out=contrib[:, 2:3], in0=bml[:], in1=vb[:])
            nc.vector.tensor_copy(out=contrib[:, 3:4], in_=vh[:])
            nc.vector.tensor_copy(out=contrib[:, 4:5], in_=vb[:])
            blk_contribs.append(contrib)

        tot = blk_contribs[0]
        nc.vector.tensor_add(out=tot[:], in0=tot[:], in1=blk_contribs[1][:])

        ps = ppool.tile([5, 1], F32, tag="ps5", name="ps5")
        nc.tensor.matmul(ps[:], lhsT=tot[:], rhs=ones[:], start=True, stop=True)
        res = smpool.tile([5, 1], F32, tag="res", name="res")
        nc.scalar.copy(out=res[:], in_=ps[:])
        nc.sync.dma_start(out=out[:], in_=res[:])


def _build(bounds):
    N0, N1 = bounds
    nc = bacc.Bacc("TRN2", target_bir_lowering=False, debug=False)
    q = nc.dram_tensor("q", [NBLK, PBLK, D], F32, kind="ExternalInput")
    k = nc.dram_tensor("k", [NBLK, PBLK, D], F32, kind="ExternalInput")
    k2 = nc.dram_tensor("k2", [NBLK, PBLK, D], F32, kind="ExternalInput")
    q16 = nc.dram_tensor("q16", [NBLK, PBLK, D], F16, kind="ExternalInput")
    hn0 = nc.dram_tensor("hn0", [PBLK, N0, D], F16, kind="ExternalInput")
    hn1 = nc.dram_tensor("hn1", [PBLK, N1, D], F16, kind="ExternalInput")
    fn = nc.dram_tensor("fn", [NBLK, PBLK, M_MAX, D], F16, kind="ExternalInput")
    hc = nc.dram_tensor("hn_counts", [NBLK, PBLK, 1], I32, kind="ExternalInput")
    fc = nc.dram_tensor("fn_counts", [NBLK, PBLK, 1], I32, kind="ExternalInput")
    out = nc.dram_tensor("out", [5, 1], F32, kind="ExternalOutput")
    with tile.TileContext(nc) as tc:
        _emit(tc, bounds, q, q16, k, k2, (hn0, hn1), fn, hc, fc, out)
    nc.compile()
    return nc


_NC_CACHE = {}


def _get_nc(bounds):
    if bounds not in _NC_CACHE:
        _NC_CACHE[bounds] = _build(bounds)
    return _NC_CACHE[bounds]


def _round8(x):
    return max(8, int(-(-int(x) // 8) * 8))


def plan(hn_counts):
    """Global count-sorted block schedule: returns (order, (N0, N1))."""
    order = np.argsort(-hn_counts, kind="stable")
    blocks = order.reshape(2 * N_CORES, PBLK)
    c = np.asarray(hn_counts)
    n0 = _round8(c[blocks[0:N_CORES]].max())
    n1 = _round8(c[blocks[N_CORES:]].max())
    return blocks, (min(n0, N_MAX), min(n1, N_MAX))


def make_in_maps(q, k, k2, hn, fn, hn_counts, fn_counts):
    q = np.asarray(q, np.float32)
    k = np.asarray(k, np.float32)
    k2 = np.asarray(k2, np.float32)
    hn = np.asarray(hn, np.float32)
    fn = np.asarray(fn, np.float32)
    hn_counts = np.asarray(hn_counts, np.int32)
    fn_counts = np.asarray(fn_counts, np.int32)
    blocks, (n0, n1) = plan(hn_counts)
    q16 = q.astype(np.float16)
    hn16 = hn.astype(np.float16)
    fn16 = fn.astype(np.float16)
    hn_v0 = hn16[:, :n0, :]   # views, no copy
    hn_v1 = hn16[:, :n1, :]
    in_maps = []
    for c in range(N_CORES):
        i0, i1 = blocks[c], blocks[N_CORES + c]
        both = np.stack([i0, i1])
        in_maps.append({
            "q": q[both],
            "q16": q16[both],
            "k": k[both],
            "k2": k2[both],
            "hn0": hn_v0[i0],
            "hn1": hn_v1[i1],
            "fn": fn16[both],
            "hn_counts": hn_counts[both][..., None],
            "fn_counts": fn_counts[both][..., None],
        })
    return in_maps, (n0, n1)


def combine_partials(results):
    parts = np.stack([np.asarray(r["out"], np.float64).reshape(5) for r in results])
    cl_s, clnb_s, bml_s, nv, nb = parts.sum(axis=0)
    n_valid = max(nv, 1.0)
    cl = cl_s / n_valid
    clnb = clnb_s / n_valid
    bml_mean = (bml_s / nb) if nb > 0 else 0.0
    lbml = LAMBDA_BML * bml_mean
    tot = cl + clnb + lbml
    return np.array([tot, cl, lbml, clnb], np.float32)


def run_spmd(in_maps, bounds, **kwargs):
    nc = _get_nc(bounds)
    return bass_utils.run_bass_kernel_spmd(
        nc, in_maps, core_ids=list(range(N_CORES)), **kwargs
    )


def kernel(q, k, k2, hn, fn, hn_counts, fn_counts):
    in_maps, bounds = make_in_maps(q, k, k2, hn, fn, hn_counts, fn_counts)
    res = run_spmd(in_maps, bounds)
    return combine_partials(res.results)


# revision 13
# speedup vs baseline: 2.0044x; 1.2824x over previous
"""ClusterLoss Bass/Tile kernel for Trainium2 (8 NeuronCores, data parallel).

Strategy
--------
Pure data parallelism over B=2048 with a count-aware schedule: samples are
globally sorted by hn_count and dealt into 16 blocks of 128; every core gets
one "big" block (slot 0, padded negative bound N0) and one "small" block
(slot 1, bound N1), so a single SPMD program with static loop bounds fits all
cores while skipping most padded negatives.  The four losses are sums over
samples, so no output unpermutation is needed.

Per block, samples sit on the 128 partitions with the feature dim d on the
free axis.  The einsum('bd,bnd->bn') runs as one fused DVE tensor_tensor
multiply per 8-negative chunk (q broadcast via a stride-0 access pattern)
with the per-negative free-axis reductions on the otherwise idle Scalar
engine (activation Copy with accum_out, folding the 1/TEMP scale).  Masked
logsumexp / cross-entropy / BML terms are small per-partition vector ops.
Each core emits 5 partial sums reduced over partitions with a ones-vector
matmul; the final scalar combine runs on host.

The program is JIT-specialized to (N0, N1) derived from the counts at call
time and cached, so repeated calls with the same raggedness profile reuse
the compiled NEFF.
"""

from contextlib import ExitStack

import numpy as np

import concourse.bass as bass
import concourse.bacc as bacc
import concourse.tile as tile
from concourse import mybir
from concourse import bass_utils

N_CORES = 8
B, D, N_MAX, M_MAX = 2048, 512, 256, 32
B_LOC = B // N_CORES          # 256 samples per core
PBLK = 128                    # partition block
NBLK = B_LOC // PBLK          # 2 slots per core
NCHUNK = 8                    # negatives per hn DMA tile / DVE multiply

TEMP, ALPHA, BETA, LAMBDA_BML = 0.07, 0.4, 0.2, 0.2
NEG = -1e30
EXP_CLAMP = -87.0             # exp(-87) underflows f32; avoids LUT extremes

F32 = mybir.dt.float32
F16 = mybir.dt.float16
BF16 = mybir.dt.bfloat16
I32 = mybir.dt.int32
AF = mybir.ActivationFunctionType
OP = mybir.AluOpType
AX = mybir.AxisListType


def _bcast_n(ap, n):
    """(128, D) AP viewed as (128, n, D) with stride-0 broadcast on n."""
    return bass.AP(tensor=ap.tensor, offset=ap.offset,
                   ap=[ap.ap[0], [0, n], ap.ap[1]])


def _emit(tc, bounds, q, q16, k, k2, hns, fn, hc, fc, out):
    nc = tc.nc
    with ExitStack() as ctx:
        hpool = ctx.enter_context(tc.tile_pool(name="hnp", bufs=6))
        fpool = ctx.enter_context(tc.tile_pool(name="fnp", bufs=2))
        qpool = ctx.enter_context(tc.tile_pool(name="qkp", bufs=2))
        mpool = ctx.enter_context(tc.tile_pool(name="med", bufs=2))
        spool = ctx.enter_context(tc.tile_pool(name="scr", bufs=3))
        smpool = ctx.enter_context(tc.tile_pool(name="sm", bufs=2))
        cpool = ctx.enter_context(tc.tile_pool(name="cst", bufs=1))
        ppool = ctx.enter_context(tc.tile_pool(name="ps", bufs=2, space="PSUM"))

        # constants
        iota_i = cpool.tile([PBLK, N_MAX], I32, tag="iota_i", name="iota_i")
        nc.gpsimd.iota(iota_i[:], pattern=[[1, N_MAX]], base=0, channel_multiplier=0)
        iota_f = cpool.tile([PBLK, N_MAX], F32, tag="iota_f", name="iota_f")
        nc.vector.tensor_copy(out=iota_f[:], in_=iota_i[:])
        ones = cpool.tile([PBLK, 1], F32, tag="ones", name="ones")
        nc.vector.memset(ones[:], 1.0)
        alpha_t = cpool.tile([PBLK, 1], F32, tag="alpha_t", name="alpha_t")
        nc.vector.memset(alpha_t[:], ALPHA)
        nbeta_t = cpool.tile([PBLK, 1], F32, tag="nbeta_t", name="nbeta_t")
        nc.vector.memset(nbeta_t[:], -BETA)

        def sm(tag, dt=F32, w=1):
            return smpool.tile([PBLK, w], dt, tag=tag, name=tag)

        blk_contribs = []
        for s in range(NBLK):
            NS = bounds[s]
            q_t = qpool.tile([PBLK, D], F32, tag="q_t", name="q_t")
            nc.sync.dma_start(out=q_t[:], in_=q[s])
            k_t = qpool.tile([PBLK, D], F32, tag="k_t", name="k_t")
            nc.sync.dma_start(out=k_t[:], in_=k[s])
            k2_t = qpool.tile([PBLK, D], F32, tag="k2_t", name="k2_t")
            nc.sync.dma_start(out=k2_t[:], in_=k2[s])
            q16_t = qpool.tile([PBLK, D], BF16, tag="q16_t", name="q16_t")
            nc.sync.dma_start(out=q16_t[:], in_=q16[s])
            hc_i = sm("hc_i", I32)
            nc.sync.dma_start(out=hc_i[:], in_=hc[s])
            fc_i = sm("fc_i", I32)
            nc.sync.dma_start(out=fc_i[:], in_=fc[s])
            hc_f = sm("hc_f")
            nc.vector.tensor_copy(out=hc_f[:], in_=hc_i[:])
            fc_f = sm("fc_f")
            nc.vector.tensor_copy(out=fc_f[:], in_=fc_i[:])

            dots = spool.tile([PBLK, D], F32, tag="dots", name="dots")
            adump = spool.tile([PBLK, D], F16, tag="adump", name="adump", bufs=1)

            def rowdot(in1, scale, accum):
                nc.vector.scalar_tensor_tensor(
                    out=dots[:], in0=q_t[:], scalar=scale, in1=in1,
                    op0=OP.mult, op1=OP.mult, accum_out=accum,
                )

            lpos = sm("lpos")
            rowdot(k_t[:], 1.0 / TEMP, lpos[:])
            lposnb = sm("lposnb")
            rowdot(k2_t[:], 1.0 / TEMP, lposnb[:])
            simpos = sm("simpos")
            rowdot(k_t[:], 1.0, simpos[:])

            # negative logits: lneg[b, n] = q.hn[b, n] / TEMP
            # DVE does one (128, 8, 512) multiply per chunk (q broadcast on
            # n, 1/TEMP folded in); per-negative free-axis reductions are
            # split R_DVE:NCHUNK-R_DVE between DVE (one multi-n
            # tensor_reduce) and the otherwise idle ScalarE (Copy+accum) so
            # both engines run near their line rate.
            lneg = mpool.tile([PBLK, N_MAX], F32, tag="lneg", name="lneg")
            for c in range(NS // NCHUNK):
                r_dve = 4 if c % 2 else 3
                n0 = c * NCHUNK
                h_t = hpool.tile([PBLK, NCHUNK, D], BF16, tag="h_t", name="h_t")
                nc.sync.dma_start(
                    out=h_t[:], in_=hns[s][:, n0:n0 + NCHUNK, :]
                )
                prod = spool.tile([PBLK, NCHUNK, D], F16, tag="prod", name="prod")
                nc.vector.tensor_mul(out=prod[:], in0=h_t[:],
                                     in1=_bcast_n(q16_t[:], NCHUNK))
                nc.vector.tensor_reduce(
                    out=lneg[:, n0:n0 + r_dve], in_=prod[:, :r_dve, :],
                    axis=AX.X, op=OP.add,
                )
                for j in range(r_dve, NCHUNK):
                    nc.scalar.activation(
                        out=adump[:], in_=prod[:, j, :], func=AF.Copy,
                        scale=1.0, accum_out=lneg[:, n0 + j:n0 + j + 1],
                    )
            # bf16 tensor_tensor cannot fold a scale; apply 1/TEMP once here
            nc.vector.tensor_scalar_mul(out=lneg[:, :NS], in0=lneg[:, :NS],
                                        scalar1=1.0 / TEMP)

            # fn dots: q.fn[b, m]; same split, no TEMP scale
            fnd = sm("fnd", w=M_MAX)
            for c in range(M_MAX // NCHUNK):
                r_dve = 4 if c % 2 else 3
                m0 = c * NCHUNK
                f_t = fpool.tile([PBLK, NCHUNK, D], BF16, tag="f_t", name="f_t")
                nc.sync.dma_start(
                    out=f_t[:], in_=fn[s, :, m0:m0 + NCHUNK, :]
                )
                prodf = spool.tile([PBLK, NCHUNK, D], F16, tag="prod", name="prodf")
                nc.vector.tensor_mul(out=prodf[:], in0=f_t[:],
                                     in1=_bcast_n(q16_t[:], NCHUNK))
                nc.vector.tensor_reduce(
                    out=fnd[:, m0:m0 + r_dve], in_=prodf[:, :r_dve, :],
                    axis=AX.X, op=OP.add,
                )
                for j in range(r_dve, NCHUNK):
                    nc.scalar.activation(
                        out=adump[:], in_=prodf[:, j, :], func=AF.Copy,
                        scale=1.0, accum_out=fnd[:, m0 + j:m0 + j + 1],
                    )

            # mask padded negatives to -1e30, then logsumexp along free axis
            mneg = mpool.tile([PBLK, N_MAX], F32, tag="mneg", name="mneg")
            nc.vector.tensor_scalar(
                out=mneg[:, :NS], in0=iota_f[:, :NS], scalar1=hc_f[:],
                scalar2=NEG, op0=OP.is_ge, op1=OP.mult,
            )
            nc.vector.tensor_add(out=lneg[:, :NS], in0=lneg[:, :NS],
                                 in1=mneg[:, :NS])
            mrow = sm("mrow")
            nc.vector.tensor_reduce(out=mrow[:], in_=lneg[:, :NS], axis=AX.X,
                                    op=OP.max)
            nmrow = sm("nmrow")
            nc.vector.tensor_scalar_mul(out=nmrow[:], in0=mrow[:], scalar1=-1.0)
            expin = mpool.tile([PBLK, N_MAX], F32, tag="expin", name="expin")
            nc.vector.tensor_scalar(
                out=expin[:, :NS], in0=lneg[:, :NS], scalar1=nmrow[:],
                scalar2=EXP_CLAMP, op0=OP.add, op1=OP.max,
            )
            expout = mpool.tile([PBLK, N_MAX], F32, tag="expout", name="expout")
            sumexp = sm("sumexp")
            nc.scalar.activation(
                out=expout[:, :NS], in_=expin[:, :NS], func=AF.Exp,
                accum_out=sumexp[:],
            )
            lse = sm("lse")
            nc.scalar.activation(out=lse[:], in_=sumexp[:], func=AF.Ln)
            nc.vector.tensor_add(out=lse[:], in0=lse[:], in1=mrow[:])

            # ce(lp) = logaddexp(lp, lse) - lp
            def ce(lp, tag):
                mm = sm("mm" + tag)
                nc.vector.tensor_max(out=mm[:], in0=lp[:], in1=lse[:])
                nmm = sm("nmm" + tag)
                nc.vector.tensor_scalar_mul(out=nmm[:], in0=mm[:], scalar1=-1.0)
                e1 = sm("e1" + tag)
                nc.vector.tensor_scalar(
                    out=e1[:], in0=lp[:], scalar1=nmm[:], scalar2=EXP_CLAMP,
                    op0=OP.add, op1=OP.max,
                )
                nc.scalar.activation(out=e1[:], in_=e1[:], func=AF.Exp)
                e2 = sm("e2" + tag)
                nc.vector.tensor_scalar(
                    out=e2[:], in0=lse[:], scalar1=nmm[:], scalar2=EXP_CLAMP,
                    op0=OP.add, op1=OP.max,
                )
                nc.scalar.activation(out=e2[:], in_=e2[:], func=AF.Exp)
                s12 = sm("s12" + tag)
                nc.vector.tensor_add(out=s12[:], in0=e1[:], in1=e2[:])
                nc.scalar.activation(out=s12[:], in_=s12[:], func=AF.Ln)
                cev = sm("ce" + tag)
                nc.vector.tensor_add(out=cev[:], in0=s12[:], in1=mm[:])
                nc.vector.tensor_sub(out=cev[:], in0=cev[:], in1=lp[:])
                return cev

            cep = ce(lpos, "p")
            cenb = ce(lposnb, "n")

            # BML term
            maskf = sm("maskf", w=M_MAX)
            nc.vector.tensor_scalar(
                out=maskf[:], in0=iota_f[:, :M_MAX], scalar1=fc_f[:],
                scalar2=None, op0=OP.is_lt,
            )
            nc.vector.tensor_mul(out=fnd[:], in0=fnd[:], in1=maskf[:])
            sfn = sm("sfn")
            nc.vector.tensor_reduce(out=sfn[:], in_=fnd[:], axis=AX.X, op=OP.add)
            den = sm("den")
            nc.vector.tensor_scalar_max(out=den[:], in0=fc_f[:], scalar1=1.0)
            rden = sm("rden")
            nc.vector.reciprocal(out=rden[:], in_=den[:])
            simfn = sm("simfn")
            nc.vector.tensor_mul(out=simfn[:], in0=sfn[:], in1=rden[:])
            delta = sm("delta")
            nc.vector.tensor_sub(out=delta[:], in0=simfn[:], in1=simpos[:])
            r1 = sm("r1")
            nc.scalar.activation(out=r1[:], in_=delta[:], func=AF.Relu,
                                 bias=alpha_t[:], scale=1.0)
            r2 = sm("r2")
            nc.scalar.activation(out=r2[:], in_=delta[:], func=AF.Relu,
                                 bias=nbeta_t[:], scale=-1.0)
            bml = sm("bml")
            nc.vector.tensor_add(out=bml[:], in0=r1[:], in1=r2[:])

            vh = sm("vh")
            nc.vector.tensor_scalar(out=vh[:], in0=hc_f[:], scalar1=0.0,
                                    scalar2=None, op0=OP.is_gt)
            vf = sm("vf")
            nc.vector.tensor_scalar(out=vf[:], in0=fc_f[:], scalar1=0.0,
                                    scalar2=None, op0=OP.is_gt)
            vb = sm("vb")
            nc.vector.tensor_mul(out=vb[:], in0=vh[:], in1=vf[:])

            contrib = smpool.tile([PBLK, 5], F32, tag="contrib", name="contrib")
            nc.vector.tensor_mul(out=contrib[:, 0:1], in0=cep[:], in1=vh[:])
            nc.vector.tensor_mul(out=contrib[:, 1:2], in0=cenb[:], in1=vh[:])
            nc.vector.tensor_mul(out=contrib[:, 2:3], in0=bml[:], in1=vb[:])
            nc.vector.tensor_copy(out=contrib[:, 3:4], in_=vh[:])
            nc.vector.tensor_copy(out=contrib[:, 4:5], in_=vb[:])
            blk_contribs.append(contrib)

        tot = blk_contribs[0]
        nc.vector.tensor_add(out=tot[:], in0=tot[:], in1=blk_contribs[1][:])

        ps = ppool.tile([5, 1], F32, tag="ps5", name="ps5")
        nc.tensor.matmul(ps[:], lhsT=tot[:], rhs=ones[:], start=True, stop=True)
        res = smpool.tile([5, 1], F32, tag="res", name="res")
        nc.scalar.copy(out=res[:], in_=ps[:])
        nc.sync.dma_start(out=out[:], in_=res[:])


def _build(bounds):
    N0, N1 = bounds
    nc = bacc.Bacc("TRN2", target_bir_lowering=False, debug=False)
    q = nc.dram_tensor("q", [NBLK, PBLK, D], F32, kind="ExternalInput")
    k = nc.dram_tensor("k", [NBLK, PBLK, D], F32, kind="ExternalInput")
    k2 = nc.dram_tensor("k2", [NBLK, PBLK, D], F32, kind="ExternalInput")
    q16 = nc.dram_tensor("q16", [NBLK, PBLK, D], BF16, kind="ExternalInput")
    hn0 = nc.dram_tensor("hn0", [PBLK, N0, D], BF16, kind="ExternalInput")
    hn1 = nc.dram_tensor("hn1", [PBLK, N1, D], BF16, kind="ExternalInput")
    fn = nc.dram_tensor("fn", [NBLK, PBLK, M_MAX, D], BF16, kind="ExternalInput")
    hc = nc.dram_tensor("hn_counts", [NBLK, PBLK, 1], I32, kind="ExternalInput")
    fc = nc.dram_tensor("fn_counts", [NBLK, PBLK, 1], I32, kind="ExternalInput")
    out = nc.dram_tensor("out", [5, 1], F32, kind="ExternalOutput")
    with tile.TileContext(nc) as tc:
        _emit(tc, bounds, q, q16, k, k2, (hn0, hn1), fn, hc, fc, out)
    nc.compile()
    return nc


_NC_CACHE = {}


def _get_nc(bounds):
    if bounds not in _NC_CACHE:
        _NC_CACHE[bounds] = _build(bounds)
    return _NC_CACHE[bounds]


def _round8(x):
    return max(8, int(-(-int(x) // 8) * 8))


def plan(hn_counts):
    """Global count-sorted block schedule: returns (order, (N0, N1))."""
    order = np.argsort(-hn_counts, kind="stable")
    blocks = order.reshape(2 * N_CORES, PBLK)
    c = np.asarray(hn_counts)
    n0 = _round8(c[blocks[0:N_CORES]].max())
    n1 = _round8(c[blocks[N_CORES:]].max())
    return blocks, (min(n0, N_MAX), min(n1, N_MAX))


def make_in_maps(q, k, k2, hn, fn, hn_counts, fn_counts):
    q = np.asarray(q, np.float32)
    k = np.asarray(k, np.float32)
    k2 = np.asarray(k2, np.float32)
    hn = np.asarray(hn, np.float32)
    fn = np.asarray(fn, np.float32)
    hn_counts = np.asarray(hn_counts, np.int32)
    fn_counts = np.asarray(fn_counts, np.int32)
    blocks, (n0, n1) = plan(hn_counts)
    import ml_dtypes
    q16 = q.astype(ml_dtypes.bfloat16)
    hn16 = hn.astype(ml_dtypes.bfloat16)
    fn16 = fn.astype(ml_dtypes.bfloat16)
    hn_v0 = hn16[:, :n0, :]   # views, no copy
    hn_v1 = hn16[:, :n1, :]
    in_maps = []
    for c in range(N_CORES):
        i0, i1 = blocks[c], blocks[N_CORES + c]
        both = np.stack([i0, i1])
        in_maps.append({
            "q": q[both],
            "q16": q16[both],
            "k": k[both],
            "k2": k2[both],
            "hn0": hn_v0[i0],
            "hn1": hn_v1[i1],
            "fn": fn16[both],
            "hn_counts": hn_counts[both][..., None],
            "fn_counts": fn_counts[both][..., None],
        })
    return in_maps, (n0, n1)


def combine_partials(results):
    parts = np.stack([np.asarray(r["out"], np.float64).reshape(5) for r in results])
    cl_s, clnb_s, bml_s, nv, nb = parts.sum(axis=0)
    n_valid = max(nv, 1.0)
    cl = cl_s / n_valid
    clnb = clnb_s / n_valid
    bml_mean = (bml_s / nb) if nb > 0 else 0.0
    lbml = LAMBDA_BML * bml_mean
    tot = cl + clnb + lbml
    return np.array([tot, cl, lbml, clnb], np.float32)


def run_spmd(in_maps, bounds, **kwargs):
    nc = _get_nc(bounds)
    return bass_utils.run_bass_kernel_spmd(
        nc, in_maps, core_ids=list(range(N_CORES)), **kwargs
    )


def kernel(q, k, k2, hn, fn, hn_counts, fn_counts):
    in_maps, bounds = make_in_maps(q, k, k2, hn, fn, hn_counts, fn_counts)
    res = run_spmd(in_maps, bounds)
    return combine_partials(res.results)
